# revision 10
# baseline (speedup 1.0000x reference)
"""Trainium2 Bass kernel for the AttentionBlock problem.

Full (unsharded) inputs in, full output out. Internally: 8-way SPMD —
data-parallel over batch (4) x query-row halves (2). Each core holds the
full 4096-token batch image (for K/V) and computes attention for its own
2048 query rows.

Layout strategy (all on-chip, no transposes needed):
  - x^T [C=256, N] channel-major in SBUF (host pre-transposes + casts bf16)
  - qT [32, 2048], kT [32, 4096] channel-major (natural matmul outputs)
  - scores computed TRANSPOSED: ST[j, i] = k_j . q_i  (lhsT=kT-slice, rhs=qT)
  - P^T = exp(ST) via ACT directly PSUM->SBUF bf16 (no max subtraction:
    scores <= ~40, exp fits fp32 comfortably)
  - attended^T[c, i] = sum_j V[j, c] P^T[j, i]  (lhsT=V token-major, rhs=P^T)
  - out[i, c] = relu((attT^T @ Wo + l*bo) / l_i) + x[i, c]
    with l_i = sum_j exp(s_ij) computed by a DVE bf16 pair-tree over j-tiles
    plus tiny PE ones-matmuls; 1/l applied as per-partition ACT scale.
"""

import numpy as np
import ml_dtypes

import concourse.bass as bass
import concourse.bacc as bacc
import concourse.mybir as mybir
import concourse.tile as tile
from concourse.bass_utils import run_bass_kernel_spmd

FP32 = mybir.dt.float32
BF16 = mybir.dt.bfloat16
AF = mybir.ActivationFunctionType
ALU = mybir.AluOpType

B, HH, WW, C = 4, 64, 64, 256
N = HH * WW            # 4096 tokens per batch element
NLOC = N // 2          # 2048 query rows per core
D = C // 8             # 32 q/k channels
NCORES = 8

IC = 512               # i-chunk (query window) per inner pass
N_IC = NLOC // IC      # 4
JT = N // 128          # 32 j-tiles
JG = JT // 2           # 16 j-groups (2 j-tiles per exp op)


def build_nc(has_bv: bool, has_bo: bool):
    nc = bacc.Bacc("TRN2", target_bir_lowering=False, debug=False)

    xt = nc.dram_tensor("xt", [C, N], BF16, kind="ExternalInput")
    xtloc = nc.dram_tensor("xtloc", [C, NLOC], BF16, kind="ExternalInput")
    xloc = nc.dram_tensor("xloc", [NLOC, C], FP32, kind="ExternalInput")
    wq = nc.dram_tensor("wq", [C, D], BF16, kind="ExternalInput")
    wk = nc.dram_tensor("wk", [C, D], BF16, kind="ExternalInput")
    wv = nc.dram_tensor("wv", [C, C], BF16, kind="ExternalInput")
    wo = nc.dram_tensor("wo", [C, C], BF16, kind="ExternalInput")
    bq = nc.dram_tensor("bq", [D, 1], FP32, kind="ExternalInput")
    bk = nc.dram_tensor("bk", [D, 1], FP32, kind="ExternalInput")
    bv = nc.dram_tensor("bv", [1, C], BF16, kind="ExternalInput")
    bo = nc.dram_tensor("bo", [1, C], BF16, kind="ExternalInput")
    out = nc.dram_tensor("out", [NLOC, C], FP32, kind="ExternalOutput")

    with tile.TileContext(nc) as tc:
        _emit(tc, nc, xt, xtloc, xloc, wq, wk, wv, wo, bq, bk, bv, bo, out,
              has_bv=has_bv, has_bo=has_bo)
    nc.finalize()
    return nc


def _emit(tc, nc, xt, xtloc, xloc, wq, wk, wv, wo, bq, bk, bv, bo, out,
          *, has_bv, has_bo):
    from contextlib import ExitStack
    with ExitStack() as ctx:
        const = ctx.enter_context(tc.tile_pool(name="const", bufs=1))
        work = ctx.enter_context(tc.tile_pool(name="work", bufs=3))
        ltree = ctx.enter_context(tc.tile_pool(name="ltree", bufs=8))
        psum = ctx.enter_context(tc.tile_pool(name="psum", bufs=1, space="PSUM"))

        # ---- persistent loads -------------------------------------------
        xt_sb = [const.tile([128, N], BF16, tag=f"xt{i}", name=f"xt{i}") for i in range(2)]
        xtl_sb = [const.tile([128, NLOC], BF16, tag=f"xtl{i}", name=f"xtl{i}") for i in range(2)]
        wq_sb = [const.tile([128, D], BF16, tag=f"wq{i}", name=f"wq{i}") for i in range(2)]
        wk_sb = [const.tile([128, D], BF16, tag=f"wk{i}", name=f"wk{i}") for i in range(2)]
        wv_sb = [const.tile([128, C], BF16, tag=f"wv{i}", name=f"wv{i}") for i in range(2)]
        wo_sb = [const.tile([128, C], BF16, tag=f"wo{i}", name=f"wo{i}") for i in range(2)]
        bq_sb = const.tile([D, 1], FP32, tag="bq", name="bq")
        bk_sb = const.tile([D, 1], FP32, tag="bk", name="bk")
        bv_sb = const.tile([1, C], BF16, tag="bv", name="bv") if has_bv else None
        bo_sb = const.tile([1, C], BF16, tag="bo", name="bo") if has_bo else None
        for i in range(2):
            nc.sync.dma_start(out=xt_sb[i][:, :], in_=xt[i * 128:(i + 1) * 128, :])
            nc.sync.dma_start(out=xtl_sb[i][:, :], in_=xtloc[i * 128:(i + 1) * 128, :])
            nc.sync.dma_start(out=wq_sb[i][:, :], in_=wq[i * 128:(i + 1) * 128, :])
            nc.sync.dma_start(out=wk_sb[i][:, :], in_=wk[i * 128:(i + 1) * 128, :])
            nc.sync.dma_start(out=wv_sb[i][:, :], in_=wv[i * 128:(i + 1) * 128, :])
            nc.sync.dma_start(out=wo_sb[i][:, :], in_=wo[i * 128:(i + 1) * 128, :])
        nc.sync.dma_start(out=bq_sb[:, :], in_=bq[:, :])
        nc.sync.dma_start(out=bk_sb[:, :], in_=bk[:, :])
        if has_bv:
            nc.sync.dma_start(out=bv_sb[:, :], in_=bv[:, :])
        if has_bo:
            nc.sync.dma_start(out=bo_sb[:, :], in_=bo[:, :])

        ones_col = const.tile([128, 1], BF16, tag="ones_col", name="ones_col")   # reduce helper
        nc.vector.memset(ones_col[:, :], 1.0)
        ones_row = None
        if has_bv:
            ones_row = const.tile([1, 128], BF16, tag="ones_row", name="ones_row")
            nc.vector.memset(ones_row[:, :], 1.0)

        # ---- kT [32, 4096], qT [32, 2048] --------------------------------
        kT = const.tile([D, N], BF16, tag="kT", name="kT")
        qT = const.tile([D, NLOC], BF16, tag="qT", name="qT")
        for s in range(N // 512):
            kp = psum.tile([D, 512], FP32, tag="kqp", name="kqp", bufs=1)
            for cc in range(2):
                nc.tensor.matmul(kp[:, :], wk_sb[cc][:, :],
                                 xt_sb[cc][:, s * 512:(s + 1) * 512],
                                 start=(cc == 0), stop=(cc == 1))
            nc.vector.tensor_scalar(out=kT[:, s * 512:(s + 1) * 512], in0=kp[:, :],
                                    scalar1=bk_sb[:, :], scalar2=0.0,
                                    op0=ALU.add, op1=ALU.max)
        for s in range(NLOC // 512):
            qp = psum.tile([D, 512], FP32, tag="kqp", name="kqp", bufs=1)
            for cc in range(2):
                nc.tensor.matmul(qp[:, :], wq_sb[cc][:, :],
                                 xtl_sb[cc][:, s * 512:(s + 1) * 512],
                                 start=(cc == 0), stop=(cc == 1))
            nc.vector.tensor_scalar(out=qT[:, s * 512:(s + 1) * 512], in0=qp[:, :],
                                    scalar1=bq_sb[:, :], scalar2=0.0,
                                    op0=ALU.add, op1=ALU.max)

        # ---- V [4096, 256] token-major -----------------------------------
        v_sb = [const.tile([128, C], BF16, tag=f"v{j}", name=f"v{j}") for j in range(JT)]
        for j in range(JT):
            cc0 = j // (JT // 2)          # which xt partition chunk holds rows
            vp = psum.tile([128, C], FP32, tag="vp", name="vp", bufs=1)
            for cc in range(2):
                nc.tensor.matmul(vp[:, :],
                                 xt_sb[cc][:, j * 128:(j + 1) * 128],
                                 wv_sb[cc][:, :],
                                 start=(cc == 0), stop=(cc == 1 and not has_bv))
            if has_bv:
                nc.tensor.matmul(vp[:, :], ones_row[:, :], bv_sb[:, :],
                                 start=False, stop=True)
            nc.vector.tensor_scalar_max(out=v_sb[j][:, :], in0=vp[:, :], scalar1=0.0)

        # ---- attention ----------------------------------------------------
        attT = [[const.tile([128, IC], BF16, tag=f"attT{ic}_{c}", name=f"attT{ic}_{c}") for c in range(2)]
                for ic in range(N_IC)]
        lrow_bf = const.tile([1, NLOC], BF16, tag="lrowbf", name="lrowbf") if has_bo else None
        lcol_ps = psum.tile([128, NLOC // 128], FP32, tag="lcol", name="lcol")  # whole-kernel

        for ic in range(N_IC):
            attp = [psum.tile([128, IC], FP32, tag=f"attp{c}", name=f"attp{c}") for c in range(2)]
            accs = [None] * 6   # DVE bf16 binary-counter partial sums
            for jg in range(JG):
                stp = psum.tile([128, 1024], FP32, tag="st", name="st")
                for t in range(2):
                    j = 2 * jg + t
                    nc.tensor.matmul(stp[:, t * 512:(t + 1) * 512],
                                     kT[:, j * 128:(j + 1) * 128],
                                     qT[:, ic * IC:(ic + 1) * IC],
                                     start=True, stop=True)
                pt = work.tile([128, 1024], BF16, tag="pt", name="pt")
                nc.scalar.activation(pt[:, :], stp[:, :], AF.Exp)
                for t in range(2):
                    j = 2 * jg + t
                    for c in range(2):
                        nc.tensor.matmul(attp[c][:, :],
                                         v_sb[j][:, c * 128:(c + 1) * 128],
                                         pt[:, t * 512:(t + 1) * 512],
                                         start=(j == 0), stop=(j == JT - 1))
                # l-partial: fold pt's two j-tiles, insert into binary counter
                cur = ltree.tile([128, IC], BF16, tag="lt", name="lt")
                nc.vector.tensor_tensor(out=cur[:, :], in0=pt[:, 0:512],
                                        in1=pt[:, 512:1024], op=ALU.add)
                lvl = 0
                while accs[lvl] is not None:
                    nxt = ltree.tile([128, IC], BF16, tag="lt", name="lt")
                    nc.vector.tensor_tensor(out=nxt[:, :], in0=cur[:, :],
                                            in1=accs[lvl][:, :], op=ALU.add)
                    accs[lvl] = None
                    cur = nxt
                    lvl += 1
                accs[lvl] = cur
            partial = accs[4]   # 16 inserts -> lives at level 4
            assert partial is not None and all(a is None for i, a in enumerate(accs) if i != 4)

            # evict attT
            for c in range(2):
                nc.vector.tensor_copy(attT[ic][c][:, :], attp[c][:, :])
            if has_bo:
                # l row: [1, IC] = ones^T @ partial (bf16, feeds the bo fold)
                lrp = psum.tile([1, IC], FP32, tag="kqp", name="lrp")
                nc.tensor.matmul(lrp[:, :], ones_col[:, :], partial[:, :],
                                 start=True, stop=True)
                nc.vector.tensor_copy(lrow_bf[:, ic * IC:(ic + 1) * IC], lrp[:, :])
            # l col slices: [128, 1] = partial_slice^T @ ones
            for t in range(IC // 128):
                it = ic * (IC // 128) + t
                nc.tensor.matmul(lcol_ps[:, it:it + 1],
                                 partial[:, t * 128:(t + 1) * 128],
                                 ones_col[:, :], start=True, stop=True)

        lcol_sb = const.tile([128, NLOC // 128], FP32, tag="lcol_sb", name="lcol_sb")
        recip_l = const.tile([128, NLOC // 128], FP32, tag="recip_l", name="recip_l")
        nc.vector.tensor_copy(lcol_sb[:, :], lcol_ps[:, :])
        nc.vector.reciprocal(recip_l[:, :], lcol_sb[:, :])

        # ---- output projection + epilogue --------------------------------
        for it in range(NLOC // 128):
            ic, t = it // (IC // 128), it % (IC // 128)
            zp = psum.tile([128, C], FP32, tag="vp", name="z", bufs=1)
            for c in range(2):
                nc.tensor.matmul(zp[:, :],
                                 attT[ic][c][:, t * 128:(t + 1) * 128],
                                 wo_sb[c][:, :],
                                 start=(c == 0), stop=(c == 1 and not has_bo))
            if has_bo:
                nc.tensor.matmul(zp[:, :],
                                 lrow_bf[:, it * 128:(it + 1) * 128],
                                 bo_sb[:, :], start=False, stop=True)
            o1 = work.tile([128, C], FP32, tag="o1", name="o1")
            nc.scalar.activation(o1[:, :], zp[:, :], AF.Relu,
                                 scale=recip_l[:, it:it + 1])
            xr = work.tile([128, C], FP32, tag="xr", name="xr")
            nc.sync.dma_start(out=xr[:, :], in_=xloc[it * 128:(it + 1) * 128, :])
            o2 = work.tile([128, C], FP32, tag="o2", name="o2")
            nc.vector.tensor_tensor(out=o2[:, :], in0=o1[:, :], in1=xr[:, :],
                                    op=ALU.add)
            nc.sync.dma_start(out=out[it * 128:(it + 1) * 128, :], in_=o2[:, :])


_NC_CACHE = {}


def _get_nc(has_bv, has_bo):
    key = (has_bv, has_bo)
    if key not in _NC_CACHE:
        _NC_CACHE[key] = build_nc(has_bv, has_bo)
    return _NC_CACHE[key]


def make_in_maps(inputs, has_bv, has_bo):
    bf = ml_dtypes.bfloat16
    x = np.asarray(inputs["inputs"], np.float32).reshape(B, N, C)
    xt_all = np.ascontiguousarray(x.transpose(0, 2, 1)).astype(bf)   # [B, C, N]
    wq = np.asarray(inputs["Wq"], np.float32).astype(bf)
    wk = np.asarray(inputs["Wk"], np.float32).astype(bf)
    wv = np.asarray(inputs["Wv"], np.float32).astype(bf)
    wo = np.asarray(inputs["Wo"], np.float32).astype(bf)
    bq = np.asarray(inputs["bq"], np.float32).reshape(D, 1)
    bk = np.asarray(inputs["bk"], np.float32).reshape(D, 1)
    bv = np.asarray(inputs["bv"], np.float32).astype(bf).reshape(1, C)
    bo = np.asarray(inputs["bo"], np.float32).astype(bf).reshape(1, C)
    in_maps = []
    for core in range(NCORES):
        b, half = core // 2, core % 2
        sl = slice(half * NLOC, (half + 1) * NLOC)
        in_maps.append({
            "xt": xt_all[b],
            "xtloc": np.ascontiguousarray(xt_all[b][:, sl]),
            "xloc": np.ascontiguousarray(x[b, sl, :]),
            "wq": wq, "wk": wk, "wv": wv, "wo": wo,
            "bq": bq, "bk": bk, "bv": bv, "bo": bo,
        })
    return in_maps


def kernel(**inputs):
    has_bv = bool(np.any(np.asarray(inputs["bv"])))
    has_bo = bool(np.any(np.asarray(inputs["bo"])))
    nc = _get_nc(has_bv, has_bo)
    in_maps = make_in_maps(inputs, has_bv, has_bo)
    res = run_bass_kernel_spmd(nc, in_maps, core_ids=list(range(NCORES)))
    full = np.empty((B, N, C), np.float32)
    for core in range(NCORES):
        b, half = core // 2, core % 2
        full[b, half * NLOC:(half + 1) * NLOC, :] = res.results[core]["out"]
    return full.reshape(B, HH, WW, C)
